# revision 33
# baseline (speedup 1.0000x reference)
"""Trainium2 Bass kernel for the AdaptiveIzhikevichNeuron problem.

Reference semantics (T=32 scan over 1M independent neurons, dt=1):
    v1 = 0.04 v^2 + 6 v + 140 - u + x_t
    spike = v1 >= 30
    v' = spike ? c : v1
    u' = (1-a) u + a b v1 + d * spike

Two device kernels:

FAST PATH -- engaged when a host-side interval-arithmetic guard proves that
every neuron spikes at t=0 and none spikes at any t>=1 (true for the
reference setup's N(0,1) inputs: v1_0 = 140 + x, then v orbits ~-88 with
>100 margin to threshold).  Under the guard the reset never fires for t>=1,
so the recurrence linearizes around the square:

    z = v + 75   (0.04 v^2 + 6 v + 140 = 0.04 z^2 - 85)
    V_t  = v1_t, accumulated in PSUM fp32 by TensorE as
           I @ s_t + I @ xs_t + I @ n_t          (identity weights only)
    s_t  = ScalarE Square reading V_{t-1} straight from PSUM; the per-step
           rescaling rides in the Square's scale/bias immediates
    n_t  = -(1-a)^{-(t-1)} u_t  (prescaled so the u-recurrence is a pure
           multiply-add with a constant scalar):
           n_{t+1} = stt(V_t * -mu' + n_t),  mu' = ab/(1-a)   [one VectorE
           scalar_tensor_tensor per group, PSUM+SBUF -> SBUF bf16]

All per-step constants fold into host-side x preprocessing (xs_t =
(1-a)^{-(t-1)} (x_t - 85), row 1 also absorbs the constant s_1) and the
ACT immediates.  The only output DMA is n_{t+1} (on the GPSIMD/SWDGE queue
so it never blocks x prefetches on the Sync queue); the host recovers
v1_t = (n_t - n_{t+1}) (1-a)^{t-1} / mu' and thresholds at 30.  The guard
budgets every quantization error in that reconstruction, so the decisions
are exact (0/33.5M mismatches vs the f32 reference).

The F=1024 free dim is split into 3 independent group pipelines (one PSUM
bank each, bank-padded).  Per group the serial loop is
  Square(538) -> s-matmul(306) -> V complete -> stt(513)/Square ...
and groups interleave on the engines, so the period lands at the ScalarE
capacity bound 3*(Fg+352cyc)/1.2GHz ~= 1650 ns/step.  ScalarE and VectorE
both run ~95% busy; measured ~72-74 us vs 142 us for the general path.

GENERAL PATH (guard fails, e.g. other parameter regimes): the original
tensor_tensor/tensor_scalar formulation with on-device reset select.

Layout: host transposes x to time-major [T, M]; pure data parallel over 8
cores, core i owns neurons [i*131072, (i+1)*131072) viewed as [128, 1024].
"""

import sys
from contextlib import ExitStack

import numpy as np

sys.path.insert(0, "/opt/trn_rl_repo")

import ml_dtypes  # noqa: E402

B, C, N, T = 16, 64, 1024, 32
M = B * C * N
N_CORES = 8
MC = M // N_CORES          # neurons per core
P = 128                    # SBUF partitions
F = MC // P                # free-dim elements per partition (1024)

_CACHE: dict = {}
_FAST_GROUPS = 3


# --------------------------------------------------------------------------
# fast path
# --------------------------------------------------------------------------

def _fast_consts(a, b, c, d):
    f32 = np.float32
    r = f32(1.0) - f32(a)                 # (1-a)
    ab = f32(a) * f32(b)
    s1 = f32(0.04) * np.square(f32(c) + f32(75.0))
    cu = f32(140.0) * ab + f32(d)         # u_1 = ab*x0 + cu; n_1 = -u_1
    mup = float(ab / r)                   # constant stt scalar
    lam = [float(r ** f32(-(t - 1))) for t in range(T + 1)]  # lam[t]=(1-a)^-(t-1)
    # ACT consts for s_{t+1} = Square(scale*Vb_t + bias): per t = 1..T-2
    scales = [float(f32(0.2) * r ** (f32(t) / 2 - 1)) for t in range(T)]
    biases = [float(f32(15.0) * r ** (-f32(t) / 2)) for t in range(T)]
    return dict(r=float(r), ab=float(ab), s1=float(s1), cu=float(cu),
                mup=mup, lam=lam, scales=scales, biases=biases)


def _build_fast(a, b, c, d, ndev=N_CORES):
    import concourse.bacc as bacc
    import concourse.tile as tile
    from concourse import mybir

    cst = _fast_consts(a, b, c, d)
    nc = bacc.Bacc("TRN2", target_bir_lowering=False, debug=False,
                   num_devices=ndev)
    bf16 = mybir.dt.bfloat16
    f32d = mybir.dt.float32
    x_ap = nc.dram_tensor("x", [T, P, F], bf16, kind="ExternalInput").ap()
    w_ap = nc.dram_tensor("wts", [P, P], bf16, kind="ExternalInput").ap()
    b_ap = nc.dram_tensor("bias", [P, T], mybir.dt.float32,
                          kind="ExternalInput").ap()
    out_ap = nc.dram_tensor("out", [T, P, F], bf16, kind="ExternalOutput").ap()

    Sq = mybir.ActivationFunctionType.Square
    Op = mybir.AluOpType

    with tile.TileContext(nc, pool_alloc_mode="queue") as tc, ExitStack() as ctx:
        wpool = ctx.enter_context(tc.tile_pool(name="wp", bufs=1))
        xpool = ctx.enter_context(tc.tile_pool(name="xp", bufs=8))
        upool = ctx.enter_context(tc.tile_pool(name="up", bufs=6))
        spool = ctx.enter_context(tc.tile_pool(name="sp", bufs=4))
        cpool = ctx.enter_context(tc.tile_pool(name="cp", bufs=1))
        psum = ctx.enter_context(tc.psum_pool(name="vp", bufs=2))

        # row 0 of x holds the host-folded init state n_1 (t=0 collapses:
        # every neuron provably spikes, so n_1 = -(ab*x0 + 140ab + d) is a
        # closed-form affine of x0 -- folded on the host like the rest of
        # the per-step constants).  Loading it first shortens the ramp.
        # split the startup loads across the two HWDGE issuers (Sync and
        # Scalar queues) so the step-1 critical loads (x1 and the n_1 init
        # state) issue in parallel instead of serializing on one queue --
        # each DMA's completion receipt costs ~2us on top of the transfer
        GRP = _FAST_GROUPS
        offs = []
        o = 0
        for g in range(GRP):
            fg = (F + GRP - 1 - g) // GRP
            fg += fg % 2  # keep slices even -> 4B-aligned bf16
            fg = min(fg, F - o)
            offs.append((o, fg))
            o += fg
        assert o == F, offs

        # per-group slices, alternating queues: each group's step-1 chain
        # starts as soon as its own slices land instead of the whole rows
        x1 = xpool.tile([P, F], bf16, tag="x", name="x1")
        uu = upool.tile([P, F], bf16, tag="uu", name="uu1")
        for g, (og, fg) in enumerate(offs):
            qa = nc.sync if g % 2 == 0 else nc.scalar
            qb = nc.scalar if g % 2 == 0 else nc.sync
            qa.dma_start(out=x1[:, og:og + fg], in_=x_ap[1][:, og:og + fg])
            qb.dma_start(out=uu[:, og:og + fg], in_=x_ap[0][:, og:og + fg])

        ident = wpool.tile([P, P], bf16, tag="ident")
        nc.scalar.dma_start(out=ident[:], in_=w_ap[:])

        # per-step Square biases (bias must be a per-partition AP for Square)
        biasT = cpool.tile([P, T], f32d, tag="biasT")
        nc.scalar.dma_start(out=biasT[:], in_=b_ap[:])

        # warm the Square activation table on a tiny tile while DMAs run
        warm = cpool.tile([P, 1], bf16, tag="warm")
        nc.vector.memset(warm[:], 0.0)
        nc.scalar.activation(warm[:], warm[:], Sq, bias=warm[:], scale=1.0)


        # G fully-independent neuron-group pipelines (F split into G slices).
        # Each group has its own x/uu/s tiles, PSUM banks, and in/out DMAs,
        # so the per-group serial loop (ACT square -> s-matmul -> V ->
        # stt -> u-matmul) pipelines without cross-group coupling.  Smaller
        # tiles shorten the per-chain latency; ScalarE throughput
        # (G * (Fg + 352cyc)) is the capacity bound.
        uslc = [uu[:, og:og + fg] for og, fg in offs]

        s = [None] * GRP  # s_1 is constant, folded into xs row 1 on the host
        for t in range(1, T):
            if t == 1:
                # step 1 needs only two terms (s_1 is host-folded), so it is
                # a single DVE add in SBUF per group -- no PE, so it starts
                # as soon as x1/n_1 land, well before the ident weights
                Vg = []
                for g, (og, fg) in enumerate(offs):
                    v1g = spool.tile([P, fg], bf16, tag=f"s{g}",
                                     name=f"v1_{g}")
                    nc.vector.tensor_tensor(v1g[:], x1[:, og:og + fg],
                                            uslc[g], op=Op.add)
                    Vg.append(v1g)
            else:
                xt = xpool.tile([P, F], bf16, tag="x", name=f"x{t}")
                nc.sync.dma_start(out=xt[:], in_=x_ap[t])

                # pad each PSUM tile to a bank so groups never share a bank
                Vg = [psum.tile([P, offs[g][1]], f32d, tag=f"V{g}",
                                name=f"V{t}_{g}", padded_shape=[P, 512])
                      for g in range(GRP)]
                if t <= 3:
                    # cold-start steps: group-major emission so group 0's V
                    # completes after 3 HAM-cold matmuls instead of 7+,
                    # staggering the group pipelines into the steady state
                    for g, (og, fg) in enumerate(offs):
                        nc.tensor.matmul(out=Vg[g][:], lhsT=ident,
                                         rhs=xt[:, og:og + fg],
                                         start=True, stop=False)
                        nc.tensor.matmul(out=Vg[g][:], lhsT=ident,
                                         rhs=uslc[g],
                                         start=False, stop=(s[g] is None))
                        if s[g] is not None:
                            nc.tensor.matmul(out=Vg[g][:], lhsT=ident,
                                             rhs=s[g][:],
                                             start=False, stop=True)
                else:
                    # steady state: pass-major so the ScalarE-gated s-pass
                    # of one group never blocks another group's matmuls
                    for g, (og, fg) in enumerate(offs):
                        nc.tensor.matmul(out=Vg[g][:], lhsT=ident,
                                         rhs=xt[:, og:og + fg],
                                         start=True, stop=False)
                    for g in range(GRP):
                        nc.tensor.matmul(out=Vg[g][:], lhsT=ident,
                                         rhs=uslc[g],
                                         start=False, stop=(s[g] is None))
                    for g in range(GRP):
                        if s[g] is not None:
                            nc.tensor.matmul(out=Vg[g][:], lhsT=ident,
                                             rhs=s[g][:],
                                             start=False, stop=True)

            uu_next = upool.tile([P, F], bf16, tag="uu", name=f"uu{t + 1}")
            for g, (og, fg) in enumerate(offs):
                if t < T - 1:
                    sg = spool.tile([P, fg], bf16, tag=f"s{g}",
                                    name=f"s{t + 1}_{g}")
                    nc.scalar.activation(sg[:], Vg[g][:], Sq,
                                         bias=biasT[:, t:t + 1],
                                         scale=cst["scales"][t])
                    s[g] = sg
                nc.vector.scalar_tensor_tensor(
                    uu_next[:, og:og + fg], Vg[g][:], -cst["mup"], uslc[g],
                    Op.mult, Op.add)
                if t == T - 1:
                    # final step: per-group HWDGE DMAs on alternating queues
                    # fire as each group's stt completes (SWDGE's ~2us
                    # completion latency would otherwise sit in the tail,
                    # and three DMAs would serialize on one queue)
                    q = nc.sync if g % 2 == 0 else nc.scalar
                    q.dma_start(out=out_ap[t][:, og:og + fg],
                                in_=uu_next[:, og:og + fg])
            if t < T - 1:
                # out-DMA on the GPSIMD (SWDGE) queue: keeps the Sync queue
                # free for x prefetches (an out-DMA waiting on this step's
                # stt would block the next step's x load behind it)
                nc.gpsimd.dma_start(out=out_ap[t], in_=uu_next[:])
            uu = uu_next
            uslc = [uu[:, og:og + fg] for og, fg in offs]

    if not nc.is_finalized():
        nc.finalize()
    return nc


def _fast_guard(x, a, b, c, d):
    """Host-side proof that the fast path is exact: interval arithmetic over
    the exact recurrence (plus device-arithmetic slack) showing all neurons
    spike at t=0 and none spike at t>=1, with margin exceeding every
    quantization error in the device pipeline + host reconstruction."""
    f = np.float64
    r = 1.0 - a
    if not (0.9 <= r <= 1.0) or b <= 0 or a <= 0:
        return False
    # bf16 rounding of the host-folded inputs is absorbed by quantizing here
    xb = x.reshape(M, T).astype(ml_dtypes.bfloat16).astype(np.float64)
    xmin = xb.min(axis=0)
    xmax = xb.max(axis=0)
    # t=0: v1 = 140 + x must spike for every neuron with margin
    if not (140.0 + xmin[0] >= 30.0 + 20.0):
        return False
    # device error slack per step (bf16 states s/uu, fp32 accum): generous
    eps_dyn = 1.0
    # host reconstruction error: v1 = (uu' - uu)/mu with uu bf16-rounded
    ab = a * b

    def fq(v):  # f(v) = 0.04 v^2 + 6 v + 140, exact on reals
        return 0.04 * v * v + 6.0 * v + 140.0

    vlo = vhi = float(c)
    u0 = ab * (140.0 + xmin[0]) + d
    u1 = ab * (140.0 + xmax[0]) + d
    ulo, uhi = min(u0, u1), max(u0, u1)
    uu_max = 0.0
    for t in range(1, T):
        # exact interval image of the quadratic (vertex at v = -75)
        cands = [fq(vlo), fq(vhi)]
        fhi = max(cands)
        flo = min(cands + ([fq(-75.0)] if vlo <= -75.0 <= vhi else []))
        v1lo = flo - uhi + xmin[t] - eps_dyn
        v1hi = fhi - ulo + xmax[t] + eps_dyn
        if not (-400.0 < v1lo and v1hi < 400.0):
            return False
        # reconstruction slack: 2 roundings of uu at magnitude |uu|
        mu_t = ab * r ** (-t)
        uu_hi = max(abs(ulo), abs(uhi)) * r ** (-(t - 1))
        uu_max = max(uu_max, uu_hi)
        eps_rec = 2.0 * (uu_hi * 2.0 ** -8 + 2.0 ** -14) / mu_t
        if not (v1hi + eps_rec <= 30.0 - 5.0):
            return False
        # no spike: v' = v1, u' = (1-a)u + ab*v1
        vlo, vhi = v1lo, v1hi
        ulo, uhi = (r * ulo + min(ab * v1lo, ab * v1hi) - eps_dyn * abs(ab),
                    r * uhi + max(ab * v1lo, ab * v1hi) + eps_dyn * abs(ab))
        if not (-300.0 < ulo and uhi < 300.0):
            return False
    # prescaled state must stay in comfortable bf16 range
    if uu_max * r ** (-(T - 1)) > 1000.0:
        return False
    return True


def _run_fast(x, a, b, c, d, _trace=False):
    from concourse.bass_utils import run_bass_kernel_spmd

    f32 = np.float32
    cst = _fast_consts(a, b, c, d)
    key = (round(a, 12), round(b, 12), round(c, 12), round(d, 12), "fast")
    if key not in _CACHE:
        _CACHE[key] = _build_fast(a, b, c, d)
    nc = _CACHE[key]

    bf16 = ml_dtypes.bfloat16
    xin = np.asarray(x)
    in_dtype = xin.dtype
    xtm = np.ascontiguousarray(xin.reshape(M, T).T).astype(f32)  # [T, M]
    xs = xtm.copy()
    xs[0] = -(f32(cst["ab"]) * xtm[0] + f32(cst["cu"]))  # n_1 init state
    xs[1:] -= f32(85.0)
    xs[1] += f32(cst["s1"])
    lam = np.array(cst["lam"], dtype=f32)
    xs[1:] *= lam[1:T, None]
    xs_b = xs.astype(bf16)

    wts_b = np.ascontiguousarray(np.eye(P, dtype=f32).astype(bf16))
    bias_h = np.ascontiguousarray(
        np.tile(np.array(cst["biases"], dtype=f32), (P, 1)))

    in_maps = []
    for i in range(N_CORES):
        xi = np.ascontiguousarray(xs_b[:, i * MC:(i + 1) * MC]).reshape(T, P, F)
        in_maps.append({"x": xi, "wts": wts_b, "bias": bias_h})
    res = run_bass_kernel_spmd(nc, in_maps, core_ids=list(range(N_CORES)),
                               trace=_trace)

    # host reconstruction: out rows t=1..31 hold n_{t+1}; n_1 replicated
    n_rows = np.concatenate(
        [np.asarray(res.results[i]["out"]).reshape(T, MC)
         for i in range(N_CORES)], axis=1).astype(f32)
    n1 = xs_b[0].astype(f32)

    spikes = np.zeros((M, T), dtype=f32)
    spikes[:, 0] = 1.0
    prev = n1
    for t in range(1, T):
        cur = n_rows[t]
        v1 = (prev - cur) / f32(cst["mup"]) / lam[t]
        spikes[:, t] = (v1 >= f32(30.0)).astype(f32)
        prev = cur
    out = spikes.reshape(B, C, N, T).astype(in_dtype, copy=False)
    if _trace:
        return out, res
    return out


# --------------------------------------------------------------------------
# general path (original kernel, unchanged)
# --------------------------------------------------------------------------

def _build_general(a: float, b: float, c: float, d: float,
                   t0_all_spike: bool = False):
    import concourse.bacc as bacc
    import concourse.tile as tile
    from concourse import mybir

    nc = bacc.Bacc("TRN2", target_bir_lowering=False, debug=False,
                   num_devices=N_CORES)
    bf16 = mybir.dt.bfloat16
    x_ap = nc.dram_tensor("x", [T, P, F], bf16, kind="ExternalInput").ap()
    out_ap = nc.dram_tensor("out", [T, P, F], bf16, kind="ExternalOutput").ap()

    f32 = np.float32
    bias_s = float(f32(f32(0.2) * f32(c) + f32(15.0)))
    one_minus_a = float(f32(1.0) - f32(a))
    ab = float(f32(a) * f32(b))
    kappa2 = float(f32((1 - a) * (c + 85.0) - a * b * c - d - 85.0 - c))
    ka = float(f32(-kappa2 / 2))
    kb = float(f32(-kappa2) - f32(ka))
    theta = float(f32(30.0) - f32(c))
    d_eff = float(d) if d != 0.0 else 1.0
    sq_scale = float(f32(f32(0.2) / f32(d_eff)))
    Sq = mybir.ActivationFunctionType.Square
    Cp = mybir.ActivationFunctionType.Copy
    Op = mybir.AluOpType

    with tile.TileContext(nc, pool_alloc_mode="queue") as tc, ExitStack() as ctx:
        state = ctx.enter_context(tc.tile_pool(name="state", bufs=4))
        xpool = ctx.enter_context(tc.tile_pool(name="xp", bufs=8))
        qpool = ctx.enter_context(tc.tile_pool(name="qp", bufs=8))
        tmp = ctx.enter_context(tc.tile_pool(name="tmp", bufs=6))

        consts = ctx.enter_context(tc.tile_pool(name="consts", bufs=1))
        bias_tile = consts.tile([P, 1], mybir.dt.float32, tag="bias_s")
        nc.vector.memset(bias_tile[:], bias_s)

        vt = Wc = None

        vt0 = float(f32(d_eff) * -f32(c))
        wc0 = float(f32(85.0) + f32(c))
        s0 = float(np.square(f32(sq_scale) * f32(vt0) + f32(bias_s)))
        w10 = float(f32(one_minus_a) * f32(wc0) + f32(ka))

        t_start = 0
        if t0_all_spike:
            s1c = float(np.square(f32(bias_s)))
            CW = float(f32(w10) + f32(kb)
                       + f32(ab) * (f32(s0) - f32(wc0)) - f32(s1c))
            ka2 = float(f32(ka) + f32(one_minus_a) * f32(s1c))
            xt0 = xpool.tile([P, F], bf16, tag="x")
            nc.sync.dma_start(out=xt0[:], in_=x_ap[0])
            Wc = state.tile([P, F], bf16, tag="Wc")
            nc.vector.tensor_scalar(Wc[:], xt0[:], ab, CW, Op.mult, Op.add)
            t_start = 1

        for t in range(t_start, T):
            last = t == T - 1
            merged = t0_all_spike and t == 1
            xt = xpool.tile([P, F], bf16, tag="x")
            nc.sync.dma_start(out=xt[:], in_=x_ap[t])

            if not merged:
                s = tmp.tile([P, F], bf16, tag="s")
                if t == 0:
                    nc.vector.memset(s[:], s0)
                else:
                    nc.scalar.activation(s[:], vt[:], Sq, bias=bias_tile[:],
                                         scale=sq_scale)

            if not last:
                w1 = tmp.tile([P, F], bf16, tag="w1")
                if t == 0:
                    nc.vector.memset(w1[:], w10)
                else:
                    nc.scalar.activation(w1[:], Wc[:], Cp,
                                         bias=ka2 if merged else ka,
                                         scale=one_minus_a)

            if merged:
                v1 = tmp.tile([P, F], bf16, tag="v1")
                nc.vector.tensor_tensor(v1[:], xt[:], Wc[:], op=Op.subtract)
            else:
                y = tmp.tile([P, F], bf16, tag="y")
                if t == 0:
                    nc.vector.tensor_scalar(y[:], xt[:], wc0, None,
                                            Op.subtract)
                else:
                    nc.vector.tensor_tensor(y[:], xt[:], Wc[:],
                                            op=Op.subtract)
                v1 = tmp.tile([P, F], bf16, tag="v1")
                nc.vector.tensor_tensor(v1[:], y[:], s[:], op=Op.add)

            qd = qpool.tile([P, F], bf16, tag="qd")
            nc.vector.tensor_scalar(qd[:], v1[:], theta, d_eff,
                                    Op.is_lt, Op.mult)
            nc.sync.dma_start(out=out_ap[t], in_=qd[:])

            if last:
                break

            v2 = tmp.tile([P, F], bf16, tag="v2")
            nc.scalar.activation(v2[:], v1[:], Cp, bias=kb, scale=ab)

            vt = state.tile([P, F], bf16, tag="vt")
            nc.vector.tensor_tensor(vt[:], v1[:], qd[:], op=Op.mult)

            if d != 0.0:
                u1 = tmp.tile([P, F], bf16, tag="u1")
                nc.vector.tensor_tensor(u1[:], w1[:], qd[:], op=Op.subtract)
            else:
                u1 = w1

            Wc = state.tile([P, F], bf16, tag="Wc")
            nc.vector.tensor_tensor(Wc[:], u1[:], v2[:], op=Op.add)
    if not nc.is_finalized():
        nc.finalize()
    return nc


def _run_general(x, a, b, c, d, _trace=False):
    from concourse.bass_utils import run_bass_kernel_spmd

    xin0 = np.asarray(x)
    t0_all_spike = bool(xin0[..., 0].min() > -100.0)
    key = (round(a, 9), round(b, 9), round(c, 9), round(d, 9), t0_all_spike)
    if key not in _CACHE:
        _CACHE[key] = _build_general(a, b, c, d, t0_all_spike)
    nc = _CACHE[key]

    xin = np.asarray(x)
    in_dtype = xin.dtype
    bf16 = ml_dtypes.bfloat16
    xtm = np.ascontiguousarray(xin.reshape(M, T).astype(bf16).T)
    in_maps = [
        {"x": np.ascontiguousarray(xtm[:, i * MC:(i + 1) * MC]).reshape(T, P, F)}
        for i in range(N_CORES)
    ]
    res = run_bass_kernel_spmd(nc, in_maps, core_ids=list(range(N_CORES)),
                               trace=_trace)
    qds = np.concatenate(
        [np.asarray(res.results[i]["out"]).reshape(T, MC) for i in range(N_CORES)],
        axis=1,
    )
    spikes = (qds == 0).astype(np.float32).T.reshape(B, C, N, T)
    if t0_all_spike:
        spikes[..., 0] = 1.0
    out = spikes.astype(in_dtype, copy=False)
    if _trace:
        return out, res
    return out


def kernel(x, a, b, c, d, _trace=False):
    a, b, c, d = (float(np.asarray(v)) for v in (a, b, c, d))
    xin = np.asarray(x)
    if _fast_guard(xin, a, b, c, d):
        return _run_fast(xin, a, b, c, d, _trace=_trace)
    return _run_general(xin, a, b, c, d, _trace=_trace)
